# revision 34
# baseline (speedup 1.0000x reference)
# Trainium2 Bass kernel for nn_MultiHeadAttention_71674414235938
#
# MHA with a cross-modal additive bias gathered from a 3x3 table and a causal
# mask, B=1, S=2048, HID=1024, H=16 heads of D=64.
#
# Sharding: tensor-parallel over heads. 2 heads per core (dq slice of 128).
# Each core computes q/k/v projections for its heads, head-local attention,
# and a partial output ctx_c @ Wo[:, c*128:(c+1)*128].T which the host sums.
#
# Device-side layout choices:
#   * scores are computed TRANSPOSED: sT[j, i] = k[j]·q[i] (j on partitions),
#     so softmax-denominators and the attn@V contraction both run without any
#     on-chip transposes:  ctxT[d, i] = sum_j v'[j, d] * attnT[j, i]  with
#     lhsT = v' (natural layout) and rhs = attnT (as produced).
#   * the 3x3 cross-modal bias is rank-3:  bias = (onehot(m) @ cmw) @ onehot(m).T
#     so it is folded into the scores matmul by appending 3 rows (U.T to the
#     q side, R.T to the k side), K = 64+3 = 67.
#   * softmax runs without max-subtraction: scores are O(+-6) here, exp is
#     safely in fp32 range.
#   * a ones-column appended to v makes the PE accumulate the softmax
#     denominator into ctxT row 64; normalization happens on the way out of
#     PSUM (reciprocal + partition-broadcast DMA + multiply).
#   * causal structure: score blocks entirely above the diagonal are skipped;
#     diagonal staircase blocks are masked multiplicatively after exp.

import math

import numpy as np
import ml_dtypes

B, S, HID, H, D = 1, 2048, 1024, 16, 64
NCORES = 8
HPC = H // NCORES          # heads per core = 2
DPC = HPC * D              # head-dim columns per core = 128
KC = HID // 128            # contraction chunks = 8
NIC = S // 512             # 512-wide i-chunks = 4
NJB = S // 128             # 128-tall j-blocks = 16

BF16 = ml_dtypes.bfloat16

_CACHE = {}


def _build(causal: bool, has_bq: bool, has_bk: bool, has_bv: bool):
    from contextlib import ExitStack

    import concourse.bass as bass
    import concourse.bacc as bacc
    import concourse.mybir as mybir
    import concourse.tile as tile

    fp32 = mybir.dt.float32
    bf16 = mybir.dt.bfloat16
    Exp = mybir.ActivationFunctionType.Exp
    Copy = mybir.ActivationFunctionType.Copy

    nc = bacc.Bacc()

    xT = nc.declare_dram_parameter("xT", [HID, S], bf16, isOutput=False)
    wqT = nc.declare_dram_parameter("wqT", [HID, DPC], bf16, isOutput=False)
    wkT = nc.declare_dram_parameter("wkT", [HID, DPC], bf16, isOutput=False)
    wvT = nc.declare_dram_parameter("wvT", [HID, DPC], bf16, isOutput=False)
    woT = nc.declare_dram_parameter("woT", [DPC, HID], bf16, isOutput=False)
    uT = nc.declare_dram_parameter("uT", [4, S], bf16, isOutput=False)
    rT = nc.declare_dram_parameter("rT", [4, S], bf16, isOutput=False)
    if has_bq:
        bq = nc.declare_dram_parameter("bq", [DPC, 1], fp32, isOutput=False)
    if has_bk:
        bk = nc.declare_dram_parameter("bk", [DPC, 1], fp32, isOutput=False)
    if has_bv:
        bv = nc.declare_dram_parameter("bv", [1, DPC], fp32, isOutput=False)
    if not causal:
        maskT = nc.declare_dram_parameter("maskT", [S, S], bf16, isOutput=False)
    out = nc.declare_dram_parameter("out", [S, HID], bf16, isOutput=True)

    with tile.TileContext(nc) as tc, ExitStack() as ctx:
        pp = ctx.enter_context(tc.tile_pool(name="persist", bufs=1))

        # -- input DMAs; critical path (wq/wk, x chunks) on the sync HWDGE
        #    queue, everything else on the gpsimd SWDGE queue
        w_sbs = {}
        for nm, src in (("q", wqT), ("k", wkT)):
            w_sb = w_sbs[nm] = pp.tile([128, KC, DPC], bf16, name=f"w{nm}_sb")
            nc.sync.dma_start(
                out=w_sb, in_=src[:, :].rearrange("(kc p) m -> p kc m", p=128)
            )
        xT_re = xT[:, :].rearrange("(kc p) n -> p kc n", p=128)
        xT_sb = []
        for kc in range(KC):
            xk = pp.tile([128, S], bf16, name=f"xk{kc}")
            nc.sync.dma_start(out=xk, in_=xT_re[:, kc, :])
            xT_sb.append(xk)
        w_sbs["v"] = pp.tile([128, KC, DPC], bf16, name="wv_sb")
        nc.gpsimd.dma_start(
            out=w_sbs["v"],
            in_=wvT[:, :].rearrange("(kc p) m -> p kc m", p=128),
        )
        wo_sb = pp.tile([128, HID], bf16)
        nc.gpsimd.dma_start(out=wo_sb, in_=woT[:, :])

        # qU / kR: per head, 67 live rows ([0:64] proj, [64:67] bias factors)
        qU = [pp.tile([67, S], bf16, name=f"qU{h}") for h in range(HPC)]
        kR = [pp.tile([67, S], bf16, name=f"kR{h}") for h in range(HPC)]
        for h in range(HPC):
            nc.gpsimd.dma_start(out=qU[h][64:67, :], in_=uT[0:3, :])
            nc.gpsimd.dma_start(out=kR[h][64:67, :], in_=rT[0:3, :])
        # v': per j-block [128, 2 heads, 65] with ones in column 64
        vp = [pp.tile([128, HPC, 65], bf16, name=f"vp{jb}") for jb in range(NJB)]
        for jb in range(NJB):
            nc.gpsimd.memset(vp[jb][:, :, 64:65], 1.0)
        # normalized transposed context, both heads, one tile per i-chunk
        ctxT = [pp.tile([128, 512], bf16, name=f"ctxT{ic}") for ic in range(NIC)]
        # staircase causal mask for a diagonal 128-col strip: keep iff f >= p
        stair = None
        if causal:
            stair = pp.tile([128, 128], bf16)
            nc.vector.memset(stair, 1.0)
            nc.gpsimd.affine_select(
                out=stair, in_=stair,
                compare_op=mybir.AluOpType.is_ge,
                fill=0.0, base=0,
                pattern=[[1, 128]],
                channel_multiplier=-1,
            )
            stair_b2 = bass.AP(
                tensor=stair.tensor, offset=stair.offset,
                ap=[stair.ap[0], [0, HPC], stair.ap[1]],
            )
        if has_bq:
            bq_sb = pp.tile([DPC, 1], fp32)
            nc.gpsimd.dma_start(out=bq_sb, in_=bq[:, :])
        if has_bk:
            bk_sb = pp.tile([DPC, 1], fp32)
            nc.gpsimd.dma_start(out=bk_sb, in_=bk[:, :])
        if has_bv:
            bv_sb = pp.tile([128, DPC], fp32)
            bv_ap = bv[:, :]
            nc.gpsimd.dma_start(
                out=bv_sb,
                in_=bass.AP(tensor=bv_ap.tensor, offset=bv_ap.offset,
                            ap=[[0, 128], bv_ap.ap[1]]),
            )

        # ------- single fully-streamed emission; one PSUM pool:
        #   q(1) + k(1) + v(1) + sc(2) + ctx(2) + out(1) = 8 banks.
        # Per i-chunk n (causal): q/k chain n -> 4 v chains -> score chunks
        # (jb <= 4n+3, ic=n) with exp streaming on ACT -> both heads' ctx
        # chains -> out-projection rows. Emission order IS the per-engine
        # execution order, so this is the pipeline schedule.
        p2 = ctx.enter_context(tc.tile_pool(name="ph2", bufs=1))
        ps = ctx.enter_context(tc.tile_pool(name="ps", bufs=1, space="PSUM"))
        at_tiles = {}
        vjb_iter = iter(range(NJB))

        def emit_qk_batch(ns, tags):
            # kc-outer over 4 chains (q/k x two n-chunks) so matmuls start
            # as soon as each x chunk lands
            chains = []
            for n in ns:
                for nm in ("q", "k"):
                    chains.append((nm, n))
            pqs = {}
            for (nm, n), tg in zip(chains, tags):
                pqs[(nm, n)] = ps.tile([128, 512], fp32, tag=tg,
                                       name=f"ps_{nm}{n}")
            for kc in range(KC):
                for nm, n in chains:
                    nc.tensor.matmul(
                        pqs[(nm, n)],
                        lhsT=w_sbs[nm][:, kc, :],
                        rhs=xT_sb[kc][:, n * 512:(n + 1) * 512],
                        start=(kc == 0),
                        stop=(kc == KC - 1),
                    )
            for nm, n in chains:
                dsts = qU if nm == "q" else kR
                bias_sb = None
                if nm == "q" and has_bq:
                    bias_sb = bq_sb
                if nm == "k" and has_bk:
                    bias_sb = bk_sb
                for h in range(HPC):
                    dst = dsts[h][0:64, n * 512:(n + 1) * 512]
                    sr = pqs[(nm, n)][h * 64:(h + 1) * 64, :]
                    if bias_sb is not None:
                        nc.vector.tensor_scalar_add(
                            dst, sr, bias_sb[h * 64:(h + 1) * 64, 0:1]
                        )
                    else:
                        nc.vector.tensor_copy(dst, sr)

        def emit_v(count):
            for vjb in [v for _, v in zip(range(count), vjb_iter)]:
                psv = ps.tile([128, DPC], fp32, tag=f"abcd"[vjb % 4],
                              name=f"psv{vjb}")
                for kc in range(KC):
                    nc.tensor.matmul(
                        psv,
                        lhsT=xT_sb[kc][:, vjb * 128:(vjb + 1) * 128],
                        rhs=w_sbs["v"][:, kc, :],
                        start=(kc == 0),
                        stop=(kc == KC - 1),
                    )
                dst = vp[vjb][:, :, 0:64]
                sr = psv[:, :].rearrange("p (h m) -> p h m", h=HPC)
                if has_bv:
                    bvr = bv_sb[:, :].rearrange("p (h m) -> p h m", h=HPC)
                    nc.vector.tensor_add(dst, sr, bvr)
                else:
                    nc.vector.tensor_copy(dst, sr)

        def emit_chunk(jb, ic):
            if causal:
                ics = (jb * 128) // 512
                w = S - ics * 512
                key = jb
            else:
                ics, w, key = ic, 512, (jb, ic)
            if key not in at_tiles:
                at_tiles[key] = p2.tile(
                    [128, HPC, w], bf16, tag=f"at{jb}",
                    bufs=1 if causal else 2, name=f"at{jb}_{ic}")
            at = at_tiles[key]
            diag = causal and ic == ics
            d0 = (jb % 4) * 128 if diag else 0
            sc = ps.tile([128, HPC * 512], fp32, tag="sc", bufs=2,
                         name=f"sc{jb}_{ic}")
            for h in range(HPC):
                nc.tensor.matmul(
                    sc[:, h * 512 + d0:(h + 1) * 512],
                    lhsT=kR[h][:, jb * 128:(jb + 1) * 128],
                    rhs=qU[h][:, ic * 512 + d0:(ic + 1) * 512],
                    start=True,
                    stop=True,
                )
            scr = sc[:, :].rearrange("p (h n) -> p h n", h=HPC)
            off = (ic - ics) * 512
            nc.scalar.activation(
                at[:, :, off + d0:off + 512], scr[:, :, d0:], Exp
            )
            if diag:
                if d0:
                    nc.gpsimd.memset(at[:, :, 0:d0], 0.0)
                nc.vector.tensor_mul(
                    at[:, :, d0:d0 + 128], at[:, :, d0:d0 + 128], stair_b2
                )
            if not causal:
                mt = p2.tile([128, 512], bf16, tag="mt", bufs=2,
                             name=f"mt{jb}_{ic}")
                nc.sync.dma_start(
                    out=mt,
                    in_=maskT[jb * 128:(jb + 1) * 128,
                              ic * 512:(ic + 1) * 512])
                mt_b2 = bass.AP(
                    tensor=mt.tensor, offset=mt.offset,
                    ap=[mt.ap[0], [0, HPC], mt.ap[1]],
                )
                nc.vector.tensor_mul(at, at, mt_b2)

        def emit_ctx(h, ic):
            jmax = (ic + 1) * 4 if causal else NJB
            cps = ps.tile([65, 512], fp32, tag="ab"[h], name=f"cps{h}_{ic}")
            for jb in range(jmax):
                if causal:
                    at = at_tiles[jb]
                    ics = (jb * 128) // 512
                    rhs = at[:, h, (ic - ics) * 512:(ic - ics + 1) * 512]
                else:
                    rhs = at_tiles[(jb, ic)][:, h, 0:512]
                nc.tensor.matmul(
                    cps,
                    lhsT=vp[jb][:, h, :],
                    rhs=rhs,
                    start=(jb == 0),
                    stop=(jb == jmax - 1),
                )
            rr = p2.tile([1, 512], fp32, tag="rr", bufs=2, name=f"rr{h}_{ic}")
            nc.vector.tensor_copy(rr, cps[64:65, :])
            rb = p2.tile([64, 512], fp32, tag="rb", bufs=2, name=f"rb{h}_{ic}")
            nc.gpsimd.partition_broadcast(rb, rr)
            nc.vector.reciprocal_approx_fast(rb, rb)
            nc.vector.tensor_mul(
                ctxT[ic][h * 64:(h + 1) * 64, :], cps[0:64, :], rb,
            )

        def emit_outproj(ib):
            ob = p2.tile([128, HID], bf16, tag="ob", bufs=3, name=f"ob{ib}")
            for oc in range(2):
                ops = ps.tile([128, 512], fp32, tag="cd"[oc],
                              name=f"ops{ib}_{oc}")
                nc.tensor.matmul(
                    ops,
                    lhsT=ctxT[ib // 4][:, (ib % 4) * 128:(ib % 4 + 1) * 128],
                    rhs=wo_sb[:, oc * 512:(oc + 1) * 512],
                    start=True,
                    stop=True,
                )
                if oc == 0:
                    nc.scalar.activation(ob[:, oc * 512:(oc + 1) * 512],
                                         ops, Copy)
                else:
                    nc.vector.tensor_copy(ob[:, oc * 512:(oc + 1) * 512], ops)
            nc.sync.dma_start(out=out[ib * 128:(ib + 1) * 128, :], in_=ob)

        if causal:
            emit_qk_batch([0, 1], ["a", "b", "c", "d"])
            for jb in range(4):
                emit_chunk(jb, 0)
            emit_qk_batch([2, 3], ["a", "b", "c", "d"])

            # interleave each phase's score chunks (ACT-paced) with the
            # v-projection chains and the previous phase's ctx/out work
            # (PE-paced) so neither engine starves
            def phase(ic, fillers):
                chunks = list(range(4 * (ic + 1))) if ic < NIC else []
                fi = list(fillers)
                n_chunks = len(chunks)
                per = max(1, (n_chunks + len(fi) - 1) // max(1, len(fi)))
                ci = 0
                while chunks or fi:
                    for _ in range(per):
                        if chunks:
                            emit_chunk(chunks.pop(0), ic)
                    if fi:
                        fi.pop(0)()
            phase(1, [lambda: emit_v(2), lambda: emit_v(2)])
            phase(2, [
                lambda: emit_v(2), lambda: emit_ctx(0, 0),
                lambda: emit_v(2), lambda: emit_ctx(1, 0),
                lambda: emit_outproj(0), lambda: emit_outproj(1),
                lambda: emit_outproj(2), lambda: emit_outproj(3),
            ])
            phase(3, [
                lambda: emit_v(2), lambda: emit_ctx(0, 1),
                lambda: emit_v(2), lambda: emit_ctx(1, 1),
                lambda: emit_outproj(4), lambda: emit_outproj(5),
                lambda: emit_outproj(6), lambda: emit_outproj(7),
                lambda: emit_v(2), lambda: emit_v(2),
                lambda: emit_ctx(0, 2), lambda: emit_ctx(1, 2),
                lambda: emit_outproj(8), lambda: emit_outproj(9),
                lambda: emit_outproj(10), lambda: emit_outproj(11),
            ])
            for h in range(HPC):
                emit_ctx(h, 3)
            for ib in range(12, 16):
                emit_outproj(ib)
        else:
            emit_qk_batch([0, 1], ["a", "b", "c", "d"])
            emit_qk_batch([2, 3], ["a", "b", "c", "d"])
            emit_v(NJB)
            for ic in range(NIC):
                for jb in range(NJB):
                    emit_chunk(jb, ic)
                for h in range(HPC):
                    emit_ctx(h, ic)
                for ib in range(4 * ic, 4 * (ic + 1)):
                    emit_outproj(ib)

    nc.compile()
    return nc



def kernel(x, Wq, bq, Wk, bk, Wv, bv, Wo, bo, cmw, mask, modality_info,
           _perf=None):
    from concourse.bass_utils import run_bass_kernel_spmd

    x = np.asarray(x, np.float32)
    Wq = np.asarray(Wq, np.float32)
    Wk = np.asarray(Wk, np.float32)
    Wv = np.asarray(Wv, np.float32)
    Wo = np.asarray(Wo, np.float32)
    bq_ = np.asarray(bq, np.float32)
    bk_ = np.asarray(bk, np.float32)
    bv_ = np.asarray(bv, np.float32)
    bo_ = np.asarray(bo, np.float32)
    cmw = np.asarray(cmw, np.float32)
    mask2 = np.asarray(mask)[0]
    mi = np.asarray(modality_info).astype(np.int64)[0]

    causal = bool(
        np.array_equal(mask2 != 0, np.tril(np.ones((S, S), bool)))
    )
    has_bq = bool(np.any(bq_))
    has_bk = bool(np.any(bk_))
    has_bv = bool(np.any(bv_))

    key = (causal, has_bq, has_bk, has_bv)
    if key not in _CACHE:
        _CACHE[key] = _build(*key)
    nc = _CACHE[key]

    scale = 1.0 / math.sqrt(D)
    # rank-3 factorization of the gathered cross-modal bias
    R = np.zeros((S, 3), np.float32)
    R[np.arange(S), mi] = 1.0
    U = R @ cmw
    uT4 = np.zeros((4, S), BF16)
    rT4 = np.zeros((4, S), BF16)
    uT4[0:3, :] = U.T.astype(BF16)
    rT4[0:3, :] = R.T.astype(BF16)
    xTb = np.ascontiguousarray(x[0].T).astype(BF16)

    in_maps = []
    for c in range(NCORES):
        sl = slice(c * DPC, (c + 1) * DPC)
        m = {
            "xT": xTb,
            # scores scale folded into the q-side weights (and bias)
            "wqT": np.ascontiguousarray(Wq[sl, :].T * scale).astype(BF16),
            "wkT": np.ascontiguousarray(Wk[sl, :].T).astype(BF16),
            "wvT": np.ascontiguousarray(Wv[sl, :].T).astype(BF16),
            "woT": np.ascontiguousarray(Wo[:, sl].T).astype(BF16),
            "uT": uT4,
            "rT": rT4,
        }
        if has_bq:
            m["bq"] = np.ascontiguousarray(bq_[sl, None] * scale)
        if has_bk:
            m["bk"] = np.ascontiguousarray(bk_[sl, None])
        if has_bv:
            m["bv"] = np.ascontiguousarray(bv_[None, sl])
        if not causal:
            m["maskT"] = np.ascontiguousarray(mask2.T != 0).astype(BF16)
        in_maps.append(m)

    res = run_bass_kernel_spmd(
        nc, in_maps, core_ids=list(range(NCORES)),
        trace=bool(_perf is not None),
    )
    outp = np.zeros((S, HID), np.float32)
    for r in res.results:
        outp += np.asarray(r["out"], dtype=np.float32)
    outp += bo_[None, :]
    if _perf is not None:
        _perf["exec_time_ns"] = res.exec_time_ns
        _perf["trace"] = res.instructions_and_trace
    return outp.reshape(B, S, HID)


# revision 36
# speedup vs baseline: 1.0105x; 1.0105x over previous
# Trainium2 Bass kernel for nn_MultiHeadAttention_71674414235938
#
# MHA with a cross-modal additive bias gathered from a 3x3 table and a causal
# mask, B=1, S=2048, HID=1024, H=16 heads of D=64.
#
# Sharding: tensor-parallel over heads. 2 heads per core (dq slice of 128).
# Each core computes q/k/v projections for its heads, head-local attention,
# and a partial output ctx_c @ Wo[:, c*128:(c+1)*128].T which the host sums.
#
# Device-side layout choices:
#   * scores are computed TRANSPOSED: sT[j, i] = k[j]·q[i] (j on partitions),
#     so softmax-denominators and the attn@V contraction both run without any
#     on-chip transposes:  ctxT[d, i] = sum_j v'[j, d] * attnT[j, i]  with
#     lhsT = v' (natural layout) and rhs = attnT (as produced).
#   * the 3x3 cross-modal bias is rank-3:  bias = (onehot(m) @ cmw) @ onehot(m).T
#     so it is folded into the scores matmul by appending 3 rows (U.T to the
#     q side, R.T to the k side), K = 64+3 = 67.
#   * softmax runs without max-subtraction: scores are O(+-6) here, exp is
#     safely in fp32 range.
#   * a ones-column appended to v makes the PE accumulate the softmax
#     denominator into ctxT row 64; normalization happens on the way out of
#     PSUM (reciprocal + partition-broadcast DMA + multiply).
#   * causal structure: score blocks entirely above the diagonal are skipped;
#     diagonal staircase blocks are masked multiplicatively after exp.

import math

import numpy as np
import ml_dtypes

B, S, HID, H, D = 1, 2048, 1024, 16, 64
NCORES = 8
HPC = H // NCORES          # heads per core = 2
DPC = HPC * D              # head-dim columns per core = 128
KC = HID // 128            # contraction chunks = 8
NIC = S // 512             # 512-wide i-chunks = 4
NJB = S // 128             # 128-tall j-blocks = 16

BF16 = ml_dtypes.bfloat16

_CACHE = {}


def _build(causal: bool, has_bq: bool, has_bk: bool, has_bv: bool):
    from contextlib import ExitStack

    import concourse.bass as bass
    import concourse.bacc as bacc
    import concourse.mybir as mybir
    import concourse.tile as tile

    fp32 = mybir.dt.float32
    bf16 = mybir.dt.bfloat16
    Exp = mybir.ActivationFunctionType.Exp
    Copy = mybir.ActivationFunctionType.Copy

    nc = bacc.Bacc()

    xT = nc.declare_dram_parameter("xT", [HID, S], bf16, isOutput=False)
    wqT = nc.declare_dram_parameter("wqT", [HID, DPC], bf16, isOutput=False)
    wkT = nc.declare_dram_parameter("wkT", [HID, DPC], bf16, isOutput=False)
    wvT = nc.declare_dram_parameter("wvT", [HID, DPC], bf16, isOutput=False)
    woT = nc.declare_dram_parameter("woT", [DPC, HID], bf16, isOutput=False)
    uT = nc.declare_dram_parameter("uT", [4, S], bf16, isOutput=False)
    rT = nc.declare_dram_parameter("rT", [4, S], bf16, isOutput=False)
    if has_bq:
        bq = nc.declare_dram_parameter("bq", [DPC, 1], fp32, isOutput=False)
    if has_bk:
        bk = nc.declare_dram_parameter("bk", [DPC, 1], fp32, isOutput=False)
    if has_bv:
        bv = nc.declare_dram_parameter("bv", [1, DPC], fp32, isOutput=False)
    if not causal:
        maskT = nc.declare_dram_parameter("maskT", [S, S], bf16, isOutput=False)
    out = nc.declare_dram_parameter("out", [S, HID], bf16, isOutput=True)

    with tile.TileContext(nc) as tc, ExitStack() as ctx:
        pp = ctx.enter_context(tc.tile_pool(name="persist", bufs=1))

        # -- input DMAs; critical path (wq/wk, x chunks) on the sync HWDGE
        #    queue, everything else on the gpsimd SWDGE queue
        w_sbs = {}
        for nm, src in (("q", wqT), ("k", wkT)):
            w_sb = w_sbs[nm] = pp.tile([128, KC, DPC], bf16, name=f"w{nm}_sb")
            nc.sync.dma_start(
                out=w_sb, in_=src[:, :].rearrange("(kc p) m -> p kc m", p=128)
            )
        xT_re = xT[:, :].rearrange("(kc p) n -> p kc n", p=128)
        xT_sb = []
        for kc in range(KC):
            xk = pp.tile([128, S], bf16, name=f"xk{kc}")
            nc.sync.dma_start(out=xk, in_=xT_re[:, kc, :])
            xT_sb.append(xk)
        w_sbs["v"] = pp.tile([128, KC, DPC], bf16, name="wv_sb")
        nc.gpsimd.dma_start(
            out=w_sbs["v"],
            in_=wvT[:, :].rearrange("(kc p) m -> p kc m", p=128),
        )
        wo_sb = pp.tile([128, HID], bf16)
        nc.gpsimd.dma_start(out=wo_sb, in_=woT[:, :])

        # qU / kR: per head, 67 live rows ([0:64] proj, [64:67] bias factors)
        qU = [pp.tile([67, S], bf16, name=f"qU{h}") for h in range(HPC)]
        kR = [pp.tile([67, S], bf16, name=f"kR{h}") for h in range(HPC)]
        for h in range(HPC):
            nc.gpsimd.dma_start(out=qU[h][64:67, :], in_=uT[0:3, :])
            nc.gpsimd.dma_start(out=kR[h][64:67, :], in_=rT[0:3, :])
        # v': per j-block [128, 2 heads, 65] with ones in column 64
        vp = [pp.tile([128, HPC, 65], bf16, name=f"vp{jb}") for jb in range(NJB)]
        for jb in range(NJB):
            nc.gpsimd.memset(vp[jb][:, :, 64:65], 1.0)
        # normalized transposed context, both heads, one tile per i-chunk
        ctxT = [pp.tile([128, 512], bf16, name=f"ctxT{ic}") for ic in range(NIC)]
        # staircase causal mask for a diagonal 128-col strip: keep iff f >= p
        stair = None
        if causal:
            stair = pp.tile([128, 128], bf16)
            nc.vector.memset(stair, 1.0)
            nc.gpsimd.affine_select(
                out=stair, in_=stair,
                compare_op=mybir.AluOpType.is_ge,
                fill=0.0, base=0,
                pattern=[[1, 128]],
                channel_multiplier=-1,
            )
            stair_b2 = bass.AP(
                tensor=stair.tensor, offset=stair.offset,
                ap=[stair.ap[0], [0, HPC], stair.ap[1]],
            )
        if has_bq:
            bq_sb = pp.tile([DPC, 1], fp32)
            nc.gpsimd.dma_start(out=bq_sb, in_=bq[:, :])
        if has_bk:
            bk_sb = pp.tile([DPC, 1], fp32)
            nc.gpsimd.dma_start(out=bk_sb, in_=bk[:, :])
        if has_bv:
            bv_sb = pp.tile([128, DPC], fp32)
            bv_ap = bv[:, :]
            nc.gpsimd.dma_start(
                out=bv_sb,
                in_=bass.AP(tensor=bv_ap.tensor, offset=bv_ap.offset,
                            ap=[[0, 128], bv_ap.ap[1]]),
            )

        # ------- single fully-streamed emission; one PSUM pool:
        #   q(1) + k(1) + v(1) + sc(2) + ctx(2) + out(1) = 8 banks.
        # Per i-chunk n (causal): q/k chain n -> 4 v chains -> score chunks
        # (jb <= 4n+3, ic=n) with exp streaming on ACT -> both heads' ctx
        # chains -> out-projection rows. Emission order IS the per-engine
        # execution order, so this is the pipeline schedule.
        p2 = ctx.enter_context(tc.tile_pool(name="ph2", bufs=1))
        ps = ctx.enter_context(tc.tile_pool(name="ps", bufs=1, space="PSUM"))
        at_tiles = {}
        vjb_iter = iter(range(NJB))

        def emit_qk_batch(ns, tags):
            # kc-outer over 4 chains (q/k x two n-chunks) so matmuls start
            # as soon as each x chunk lands
            chains = []
            for n in ns:
                for nm in ("q", "k"):
                    chains.append((nm, n))
            pqs = {}
            for (nm, n), tg in zip(chains, tags):
                pqs[(nm, n)] = ps.tile([128, 512], fp32, tag=tg,
                                       name=f"ps_{nm}{n}")
            for kc in range(KC):
                for nm, n in chains:
                    nc.tensor.matmul(
                        pqs[(nm, n)],
                        lhsT=w_sbs[nm][:, kc, :],
                        rhs=xT_sb[kc][:, n * 512:(n + 1) * 512],
                        start=(kc == 0),
                        stop=(kc == KC - 1),
                    )
            for nm, n in chains:
                dsts = qU if nm == "q" else kR
                bias_sb = None
                if nm == "q" and has_bq:
                    bias_sb = bq_sb
                if nm == "k" and has_bk:
                    bias_sb = bk_sb
                for h in range(HPC):
                    dst = dsts[h][0:64, n * 512:(n + 1) * 512]
                    sr = pqs[(nm, n)][h * 64:(h + 1) * 64, :]
                    if bias_sb is not None:
                        nc.vector.tensor_scalar_add(
                            dst, sr, bias_sb[h * 64:(h + 1) * 64, 0:1]
                        )
                    else:
                        nc.vector.tensor_copy(dst, sr)

        def emit_v(count):
            for vjb in [v for _, v in zip(range(count), vjb_iter)]:
                psv = ps.tile([128, DPC], fp32, tag=f"abcd"[vjb % 4],
                              name=f"psv{vjb}")
                for kc in range(KC):
                    nc.tensor.matmul(
                        psv,
                        lhsT=xT_sb[kc][:, vjb * 128:(vjb + 1) * 128],
                        rhs=w_sbs["v"][:, kc, :],
                        start=(kc == 0),
                        stop=(kc == KC - 1),
                    )
                dst = vp[vjb][:, :, 0:64]
                sr = psv[:, :].rearrange("p (h m) -> p h m", h=HPC)
                if has_bv:
                    bvr = bv_sb[:, :].rearrange("p (h m) -> p h m", h=HPC)
                    nc.vector.tensor_add(dst, sr, bvr)
                else:
                    nc.vector.tensor_copy(dst, sr)

        def emit_chunk(jb, ic):
            if causal:
                ics = (jb * 128) // 512
                w = S - ics * 512
                key = jb
            else:
                ics, w, key = ic, 512, (jb, ic)
            if key not in at_tiles:
                at_tiles[key] = p2.tile(
                    [128, HPC, w], bf16, tag=f"at{jb}",
                    bufs=1 if causal else 2, name=f"at{jb}_{ic}")
            at = at_tiles[key]
            diag = causal and ic == ics
            d0 = (jb % 4) * 128 if diag else 0
            sc = ps.tile([128, HPC * 512], fp32, tag="sc", bufs=2,
                         name=f"sc{jb}_{ic}")
            for h in range(HPC):
                nc.tensor.matmul(
                    sc[:, h * 512 + d0:(h + 1) * 512],
                    lhsT=kR[h][:, jb * 128:(jb + 1) * 128],
                    rhs=qU[h][:, ic * 512 + d0:(ic + 1) * 512],
                    start=True,
                    stop=True,
                )
            scr = sc[:, :].rearrange("p (h n) -> p h n", h=HPC)
            off = (ic - ics) * 512
            nc.scalar.activation(
                at[:, :, off + d0:off + 512], scr[:, :, d0:], Exp
            )
            if diag:
                if d0:
                    nc.gpsimd.memset(at[:, :, 0:d0], 0.0)
                nc.vector.tensor_mul(
                    at[:, :, d0:d0 + 128], at[:, :, d0:d0 + 128], stair_b2
                )
            if not causal:
                mt = p2.tile([128, 512], bf16, tag="mt", bufs=2,
                             name=f"mt{jb}_{ic}")
                nc.sync.dma_start(
                    out=mt,
                    in_=maskT[jb * 128:(jb + 1) * 128,
                              ic * 512:(ic + 1) * 512])
                mt_b2 = bass.AP(
                    tensor=mt.tensor, offset=mt.offset,
                    ap=[mt.ap[0], [0, HPC], mt.ap[1]],
                )
                nc.vector.tensor_mul(at, at, mt_b2)

        def emit_ctx(h, ic):
            jmax = (ic + 1) * 4 if causal else NJB
            cps = ps.tile([65, 512], fp32, tag="ab"[h], name=f"cps{h}_{ic}")
            for jb in range(jmax):
                if causal:
                    at = at_tiles[jb]
                    ics = (jb * 128) // 512
                    rhs = at[:, h, (ic - ics) * 512:(ic - ics + 1) * 512]
                else:
                    rhs = at_tiles[(jb, ic)][:, h, 0:512]
                nc.tensor.matmul(
                    cps,
                    lhsT=vp[jb][:, h, :],
                    rhs=rhs,
                    start=(jb == 0),
                    stop=(jb == jmax - 1),
                )
            rr = p2.tile([1, 512], fp32, tag="rr", bufs=2, name=f"rr{h}_{ic}")
            nc.vector.tensor_copy(rr, cps[64:65, :])
            rb = p2.tile([64, 512], fp32, tag="rb", bufs=2, name=f"rb{h}_{ic}")
            nc.gpsimd.partition_broadcast(rb, rr)
            nc.vector.reciprocal_approx_fast(rb, rb)
            nc.vector.tensor_mul(
                ctxT[ic][h * 64:(h + 1) * 64, :], cps[0:64, :], rb,
            )

        def emit_outproj(ib, use_sc=False):
            ob = p2.tile([128, HID], bf16, tag="ob", bufs=3, name=f"ob{ib}")
            for oc in range(2):
                # the last out-projections reuse the freed score-psum slots
                tg = "sc" if use_sc else "cd"[oc]
                ops = ps.tile([128, 512], fp32, tag=tg,
                              bufs=2 if use_sc else 1,
                              name=f"ops{ib}_{oc}")
                nc.tensor.matmul(
                    ops,
                    lhsT=ctxT[ib // 4][:, (ib % 4) * 128:(ib % 4 + 1) * 128],
                    rhs=wo_sb[:, oc * 512:(oc + 1) * 512],
                    start=True,
                    stop=True,
                )
                if oc == 0:
                    nc.scalar.activation(ob[:, oc * 512:(oc + 1) * 512],
                                         ops, Copy)
                else:
                    nc.vector.tensor_copy(ob[:, oc * 512:(oc + 1) * 512], ops)
            nc.sync.dma_start(out=out[ib * 128:(ib + 1) * 128, :], in_=ob)

        if causal:
            emit_qk_batch([0, 1], ["a", "b", "c", "d"])
            for jb in range(4):
                emit_chunk(jb, 0)
            emit_qk_batch([2, 3], ["a", "b", "c", "d"])

            # interleave each phase's score chunks (ACT-paced) with the
            # v-projection chains and the previous phase's ctx/out work
            # (PE-paced) so neither engine starves
            def phase(ic, fillers):
                chunks = list(range(4 * (ic + 1))) if ic < NIC else []
                fi = list(fillers)
                n_chunks = len(chunks)
                per = max(1, (n_chunks + len(fi) - 1) // max(1, len(fi)))
                ci = 0
                while chunks or fi:
                    for _ in range(per):
                        if chunks:
                            emit_chunk(chunks.pop(0), ic)
                    if fi:
                        fi.pop(0)()
            phase(1, [lambda: emit_v(2), lambda: emit_v(2)])
            phase(2, [
                lambda: emit_v(2), lambda: emit_ctx(0, 0),
                lambda: emit_v(2), lambda: emit_ctx(1, 0),
                lambda: emit_outproj(0), lambda: emit_outproj(1),
                lambda: emit_outproj(2), lambda: emit_outproj(3),
            ])
            phase(3, [
                lambda: emit_v(2), lambda: emit_ctx(0, 1),
                lambda: emit_v(2), lambda: emit_ctx(1, 1),
                lambda: emit_outproj(4), lambda: emit_outproj(5),
                lambda: emit_outproj(6), lambda: emit_outproj(7),
                lambda: emit_v(2), lambda: emit_v(2),
                lambda: emit_ctx(0, 2), lambda: emit_ctx(1, 2),
                lambda: emit_outproj(8), lambda: emit_outproj(9),
                lambda: emit_outproj(10), lambda: emit_outproj(11),
            ])
            for h in range(HPC):
                emit_ctx(h, 3)
            for ib in range(12, 16):
                emit_outproj(ib, use_sc=True)
        else:
            emit_qk_batch([0, 1], ["a", "b", "c", "d"])
            emit_qk_batch([2, 3], ["a", "b", "c", "d"])
            emit_v(NJB)
            for ic in range(NIC):
                for jb in range(NJB):
                    emit_chunk(jb, ic)
                for h in range(HPC):
                    emit_ctx(h, ic)
                for ib in range(4 * ic, 4 * (ic + 1)):
                    emit_outproj(ib)

    nc.compile()
    return nc



def kernel(x, Wq, bq, Wk, bk, Wv, bv, Wo, bo, cmw, mask, modality_info,
           _perf=None):
    from concourse.bass_utils import run_bass_kernel_spmd

    x = np.asarray(x, np.float32)
    Wq = np.asarray(Wq, np.float32)
    Wk = np.asarray(Wk, np.float32)
    Wv = np.asarray(Wv, np.float32)
    Wo = np.asarray(Wo, np.float32)
    bq_ = np.asarray(bq, np.float32)
    bk_ = np.asarray(bk, np.float32)
    bv_ = np.asarray(bv, np.float32)
    bo_ = np.asarray(bo, np.float32)
    cmw = np.asarray(cmw, np.float32)
    mask2 = np.asarray(mask)[0]
    mi = np.asarray(modality_info).astype(np.int64)[0]

    causal = bool(
        np.array_equal(mask2 != 0, np.tril(np.ones((S, S), bool)))
    )
    has_bq = bool(np.any(bq_))
    has_bk = bool(np.any(bk_))
    has_bv = bool(np.any(bv_))

    key = (causal, has_bq, has_bk, has_bv)
    if key not in _CACHE:
        _CACHE[key] = _build(*key)
    nc = _CACHE[key]

    scale = 1.0 / math.sqrt(D)
    # rank-3 factorization of the gathered cross-modal bias
    R = np.zeros((S, 3), np.float32)
    R[np.arange(S), mi] = 1.0
    U = R @ cmw
    uT4 = np.zeros((4, S), BF16)
    rT4 = np.zeros((4, S), BF16)
    uT4[0:3, :] = U.T.astype(BF16)
    rT4[0:3, :] = R.T.astype(BF16)
    xTb = np.ascontiguousarray(x[0].T).astype(BF16)

    in_maps = []
    for c in range(NCORES):
        sl = slice(c * DPC, (c + 1) * DPC)
        m = {
            "xT": xTb,
            # scores scale folded into the q-side weights (and bias)
            "wqT": np.ascontiguousarray(Wq[sl, :].T * scale).astype(BF16),
            "wkT": np.ascontiguousarray(Wk[sl, :].T).astype(BF16),
            "wvT": np.ascontiguousarray(Wv[sl, :].T).astype(BF16),
            "woT": np.ascontiguousarray(Wo[:, sl].T).astype(BF16),
            "uT": uT4,
            "rT": rT4,
        }
        if has_bq:
            m["bq"] = np.ascontiguousarray(bq_[sl, None] * scale)
        if has_bk:
            m["bk"] = np.ascontiguousarray(bk_[sl, None])
        if has_bv:
            m["bv"] = np.ascontiguousarray(bv_[None, sl])
        if not causal:
            m["maskT"] = np.ascontiguousarray(mask2.T != 0).astype(BF16)
        in_maps.append(m)

    res = run_bass_kernel_spmd(
        nc, in_maps, core_ids=list(range(NCORES)),
        trace=bool(_perf is not None),
    )
    outp = np.zeros((S, HID), np.float32)
    for r in res.results:
        outp += np.asarray(r["out"], dtype=np.float32)
    outp += bo_[None, :]
    if _perf is not None:
        _perf["exec_time_ns"] = res.exec_time_ns
        _perf["trace"] = res.instructions_and_trace
    return outp.reshape(B, S, HID)
